# revision 26
# baseline (speedup 1.0000x reference)
"""BankedLinear (MoE-style banked linear) Trainium2 Bass kernel.

Math: out[n] = sum_k bank_weights[n,k] * (tensor[n] @ W[sel[n,k]] + bias[sel[n,k]])
Shapes: tensor [8192,128] f32, bank_weights [8192,2] f32, bank_selections [8192,2] int,
        weights [64,128,128] f32, bias [64,128] f32 -> out [8192,128] f32.

Strategy (expert parallel: 8 banks per core, host-routed, memory-roofline):
  - The 64 banks are ranked by selection count and dealt rank r -> core r%8
    so the per-local-slot capacities (max over cores, baked into the single
    SPMD program) stay near the mean.  Within a core, banks are packed into
    4 output chunks whose widths are multiples of 128 (scatter constraint)
    by an exhaustive minimum-padding search.
  - The host routes each (token, k) pair to the core owning its bank and
    builds, per core, an x^T panel [128, CT] in bf16 whose columns are the
    token rows in bank-sorted slot order, plus a [128, 8*128+8] bf16 weight
    panel (lhsT layout, bias tail).
  - Device, per core: weight panel in via the Pool/SWDGE DMA path, x^T in
    via 4 HWDGE slices (small first slice so the PE starts early, small
    last slice for a short tail), one bf16 matmul per bank into its own
    PSUM bank, PSUM->SBUF copies that add the bank bias and downcast to
    bf16 (split over ACT and DVE), y^T chunks out via dma_scatter_add
    descriptors prepared at program start and fired by trigger_dma as each
    chunk's copies land (outputs are pre-zeroed, so scatter-add == store).
    Dummy matmuls warm the PE p-state so real matmuls run at full clock.
  - The host finishes with out[n] = sum_k bw[n,k] * Y[core(n,k)][slot(n,k)],
    a pure gather+FMA over the returned panels.
"""

import itertools
import numpy as np
import ml_dtypes

N, K, IN, OUT, NUM_BANKS = 8192, 2, 128, 128, 64
NCORES = 8
BPC = NUM_BANKS // NCORES   # banks per core
PSUM_FREE = 512             # f32 columns per PSUM bank
BF16 = ml_dtypes.bfloat16

CFG = {
    "chunk_shape": (3, 2, 2, 1),   # banks per output chunk (desc cap order)
    "wb_path": "pool",             # 'pool' (SWDGE) or 'act' (HWDGE)
    "xs_split": "xfine",           # 'xfine'|'fine'|'half' input slicing
    "n_dummy": 11,                 # PE warm-up matmuls
    "dummy_w": 256,                # columns per warm-up matmul
    "first_seg": 128,              # split of the first bank's matmul/copy
    "pool_copies": True,           # use Pool as a third copy engine
    "pool_avail": 2900.0,          # ns when Pool frees up (after preps)
    "mm0_start": 3100.0,           # est. first real matmul start (ns)
}


def _routing_plan(sel_all):
    """Returns (group [BPC, NCORES] bank ids in slot order, caps [BPC],
    offs [BPC], CT, chunks [(c0,w,nbanks)], pair_core [N,K], pair_slot [N,K],
    xs_idx [NCORES, CT])."""
    shape = CFG["chunk_shape"]
    sel = np.asarray(sel_all).astype(np.int64)           # [N, K]
    flat = sel.reshape(-1)
    counts = np.bincount(flat, minlength=NUM_BANKS)
    order = np.argsort(-counts, kind="stable")
    group0 = order.reshape(BPC, NCORES)                  # [j, c], cap desc in j
    caps0 = counts[group0].max(axis=1).astype(np.int64)  # [BPC] desc

    # pack local banks into chunks (widths multiple of 128, min padding)
    best = None
    idxs = list(range(BPC))
    for c0 in itertools.combinations(idxs, shape[0]):
        r0 = [i for i in idxs if i not in c0]
        for c1 in itertools.combinations(r0, shape[1]):
            r1 = [i for i in r0 if i not in c1]
            for c2 in itertools.combinations(r1, shape[2]):
                c3 = tuple(i for i in r1 if i not in c2)
                parts = (c0, c1, c2, c3)
                ws = [int(-(-sum(int(caps0[i]) for i in p) // 128) * 128)
                      for p in parts]
                pad = sum(ws) - int(caps0.sum())
                key = (pad, ws[-1], -ws[0])
                if best is None or key < best[0]:
                    best = (key, parts, ws)
    _, parts, ws = best

    # final slot order: chunk by chunk, caps desc inside each chunk;
    # chunk padding goes to the last bank of the chunk
    new_order = []
    caps = []
    for p, w in zip(parts, ws):
        mem = sorted(p, key=lambda i: -caps0[i])
        new_order.extend(mem)
        cs = [int(caps0[i]) for i in mem]
        cs[-1] += w - sum(cs)
        caps.extend(cs)
    group = group0[new_order]                            # [BPC, NCORES]
    caps = np.asarray(caps, dtype=np.int64)
    offs = np.concatenate([[0], np.cumsum(caps)[:-1]]).astype(np.int64)
    CT = int(caps.sum())
    chunks = []
    j = 0
    for p, w in zip(parts, ws):
        chunks.append((int(offs[j]), int(w), len(p)))
        j += len(p)

    bank_core = np.empty(NUM_BANKS, np.int64)
    bank_local = np.empty(NUM_BANKS, np.int64)
    for j in range(BPC):
        for c in range(NCORES):
            bank_core[group[j, c]] = c
            bank_local[group[j, c]] = j

    # slot assignment: pairs sorted by bank, FIFO within bank
    sort = np.argsort(flat, kind="stable")
    starts = np.concatenate([[0], np.cumsum(counts)[:-1]])
    rank = np.arange(N * K, dtype=np.int64) - starts[flat[sort]]
    slot_sorted = offs[bank_local[flat[sort]]] + rank
    pair_slot = np.empty(N * K, np.int64)
    pair_slot[sort] = slot_sorted
    pair_core = bank_core[flat]
    tok_of_pair = np.repeat(np.arange(N, dtype=np.int64), K)

    xs_idx = np.full((NCORES, CT), N, dtype=np.int64)    # N = zero pad row
    xs_idx[pair_core, pair_slot] = tok_of_pair
    return (group, caps, offs, CT, chunks,
            pair_core.reshape(N, K), pair_slot.reshape(N, K), xs_idx)


def _build_program(caps, offs, CT, chunks):
    import concourse.bacc as bacc
    import concourse.tile as tile
    from concourse import mybir
    from concourse.tile import add_dep_helper

    f32 = mybir.dt.float32
    bf16 = mybir.dt.bfloat16
    i16 = mybir.dt.int16
    Identity = mybir.ActivationFunctionType.Identity
    NCHUNK = len(chunks)

    nc = bacc.Bacc(None, target_bir_lowering=False, debug=False)

    xs_d = nc.declare_dram_parameter("xs", [IN, CT], bf16, isOutput=False)
    wb_d = nc.declare_dram_parameter("wb", [IN, BPC * OUT + BPC], bf16,
                                     isOutput=False)
    y_ds = [nc.declare_dram_parameter(f"y{i}", [128, w], bf16, isOutput=True)
            for i, (c0, w, nb) in enumerate(chunks)]

    # chunk id for each local bank
    bank_chunk = []
    for i, (c0, w, nb) in enumerate(chunks):
        bank_chunk.extend([i] * nb)

    with tile.TileContext(nc) as tc:
        with (
            tc.tile_pool(name="const", bufs=1) as cpool,
            tc.tile_pool(name="psum", bufs=8, space="PSUM") as pspool,
        ):
            xs_sb = cpool.tile([IN, CT], bf16, tag="xs")
            wb_sb = cpool.tile([IN, BPC * OUT + BPC], bf16, tag="wb")
            ys_sbs = [cpool.tile([128, w], bf16, tag=f"ys{i}",
                                 name=f"ys{i}")
                      for i, (c0, w, nb) in enumerate(chunks)]
            bias32 = cpool.tile([128, BPC], f32, tag="bias32")
            yidx = cpool.tile([128, 8], i16, tag="yidx")
            warm = cpool.tile([128, 1], f32, tag="warm")
            junk = cpool.tile([128, max(CFG["dummy_w"], 128)], bf16,
                              tag="junk")

            # warm the ACT Identity LUT + PE junk operands during DMA head
            nc.vector.memset(warm[:], 0.0)
            nc.scalar.activation(warm[:], warm[:], Identity)
            nc.vector.memset(junk[:], 0.0)

            # weight panel via the Pool/SWDGE path (keeps HWDGE for xs)
            if CFG["wb_path"] == "pool":
                nc.gpsimd.dma_start(out=wb_sb[:], in_=wb_d.ap())
            else:
                nc.scalar.dma_start(out=wb_sb[:], in_=wb_d.ap())

            # scatter indices 0..127 (partition p -> DRAM row p, same for
            # every chunk); wrapped layout puts index v at [v%16, v//16] and
            # only the first 16 partitions carry values
            nc.gpsimd.memset(yidx[:], 0)
            nc.gpsimd.iota(yidx[:16, :], pattern=[[16, 8]],
                           base=0, channel_multiplier=1,
                           allow_small_or_imprecise_dtypes=True)

            # x^T input slices (HWDGE via SP): small first and last
            if CFG["xs_split"] == "xfine":
                f = CFG["first_seg"]
                bounds = [(0, f), (f, chunks[2][0]),
                          (chunks[2][0], chunks[3][0]),
                          (chunks[3][0], CT)]
            elif CFG["xs_split"] == "fine":
                b1 = int(caps[0])
                bounds = [(0, b1),
                          (b1, chunks[2][0]),
                          (chunks[2][0], chunks[3][0]),
                          (chunks[3][0], CT)]
            else:
                h = chunks[2][0]
                bounds = [(0, h), (h, CT)]
            for (a, b) in bounds:
                nc.sync.dma_start(out=xs_sb[:, a:b], in_=xs_d.ap()[:, a:b])

            # f32 bias columns from the bf16 panel tail
            nc.scalar.activation(bias32[:], wb_sb[:, BPC * OUT:], Identity)

            # prepared scatter descriptors for the y chunks, FIFO order
            dsem = nc.alloc_semaphore("ydma")
            csem = nc.alloc_semaphore("ycopy")
            preps = []
            for i, (c0, w, nb) in enumerate(chunks):
                p = nc.gpsimd.dma_scatter_add(
                    out_ap=y_ds[i].ap(),
                    in_ap=ys_sbs[i][:].rearrange("p (a w) -> p a w", a=1),
                    idxs_ap=yidx[:],
                    num_idxs=128, num_idxs_reg=128, elem_size=w,
                    prepare_only=True, sem=dsem, single_packet=True,
                )
                if preps:
                    add_dep_helper(p.ins, preps[-1].ins, sync=False,
                                   reason="prep FIFO order")
                preps.append(p)

            # PE warm-up: dummy matmuls so real ones run at full p-state
            dummy_ps = pspool.tile([128, PSUM_FREE], f32, tag="ps")
            dw = CFG["dummy_w"]
            for _ in range(CFG["n_dummy"]):
                nc.tensor.matmul(out=dummy_ps[:, :dw],
                                 lhsT=junk[:, :128], rhs=junk[:, :dw],
                                 start=True, stop=True)

            # segments: (bank j, col offset within bank, width); the first
            # bank is split so the first copy can begin sooner
            segs = []
            for j in range(BPC):
                cj = int(caps[j])
                f = CFG["first_seg"]
                if j == 0 and CFG["xs_split"] == "xfine" and 0 < f < cj:
                    segs.append((j, 0, f))
                    segs.append((j, f, cj - f))
                else:
                    segs.append((j, 0, cj))

            # greedy copy-engine choice by modeled completion time
            ACT, DVE, POOL = 0, 1, 2
            eng_rate = {ACT: 0.833, DVE: 1.04, POOL: 0.86}
            eng_fix = {ACT: 145, DVE: 130, POOL: 15}
            avail = {ACT: CFG["mm0_start"], DVE: CFG["mm0_start"],
                     POOL: CFG["pool_avail"]}
            engines = [ACT, DVE] + ([POOL] if CFG["pool_copies"] else [])
            mm_t = CFG["mm0_start"]
            eng_last = {ACT: None, DVE: None, POOL: None}

            copies = [[] for _ in range(NCHUNK)]
            for (j, so, w) in segs:
                oj = int(offs[j]) + so
                ci = bank_chunk[j]
                co = oj - chunks[ci][0]           # offset inside chunk tile
                pt = pspool.tile([128, PSUM_FREE], f32, tag="ps")
                nc.tensor.matmul(
                    out=pt[:, :w],
                    lhsT=wb_sb[:, j * OUT:(j + 1) * OUT],
                    rhs=xs_sb[:, oj:oj + w],
                    start=True, stop=True,
                )
                mm_t += w * 0.417
                best, bt = None, None
                for e in engines:
                    fin = max(avail[e], mm_t) + eng_fix[e] + w * eng_rate[e]
                    if bt is None or fin < bt:
                        best, bt = e, fin
                avail[best] = bt
                if best == ACT:
                    cp = nc.scalar.activation(
                        ys_sbs[ci][:, co:co + w], pt[:, :w], Identity,
                        bias=bias32[:, j:j + 1])
                elif best == DVE:
                    cp = nc.vector.tensor_scalar_add(
                        ys_sbs[ci][:, co:co + w], pt[:, :w],
                        bias32[:, j:j + 1])
                else:
                    cp = nc.gpsimd.tensor_scalar_add(
                        ys_sbs[ci][:, co:co + w], pt[:, :w],
                        bias32[:, j:j + 1])
                copies[ci].append(cp)
                eng_last[best] = cp

            # ACT/DVE's LAST copies signal csem at engine completion (their
            # in-order pipelines cover the earlier ones); a Pool drain covers
            # the Pool copies and, transitively, the prep desc-gen commits.
            n_sig = 0
            BISECT_NO_INC = True
            for e in (ACT, DVE):
                if eng_last[e] is not None and not BISECT_NO_INC:
                    eng_last[e].then_inc(csem, 1)
                    n_sig += 1

            # fire all prepared scatters; manual sem waits, nosync edges only
            # to pin the trigger after all Pool-engine work (a hoisted
            # trigger would deadlock the Pool SEQ against its own copies)
            dr = nc.gpsimd.drain(fusable=False)
            w2 = nc.gpsimd.wait_ge(csem, n_sig)
            add_dep_helper(dr.ins, preps[-1].ins, sync=False,
                           reason="trigger after preps in pool order")
            for cl in copies:
                for cp in cl:
                    add_dep_helper(dr.ins, cp.ins, sync=False,
                                   reason="trigger after pool copies")
            add_dep_helper(w2.ins, dr.ins, sync=False, reason="wait order")
            prev = w2
            for i in range(NCHUNK):
                trig = nc.gpsimd.trigger_dma(count=1)
                add_dep_helper(trig.ins, prev.ins, sync=False,
                               reason="trigger order")
                prev = trig
            # make program end wait for the scatter DMAs to land
            fw = nc.gpsimd.wait_ge(dsem, 16 * NCHUNK)
            add_dep_helper(fw.ins, prev.ins, sync=False,
                           reason="flush after trigger")

    return nc


def _make_in_maps(tensor, bank_weights, bank_selections, weights, bias,
                  group, caps, offs, CT, xs_idx):
    tensor = np.ascontiguousarray(tensor, dtype=np.float32)
    weights = np.ascontiguousarray(weights, dtype=np.float32)
    bias = np.ascontiguousarray(bias, dtype=np.float32)
    xa = np.vstack([tensor, np.zeros((1, IN), np.float32)])  # row N = pad
    in_maps = []
    for c in range(NCORES):
        banks = group[:, c]                              # [BPC]
        xsT = np.ascontiguousarray(
            xa[xs_idx[c]].T.astype(BF16))                # [128, CT]
        wb = np.empty((IN, BPC * OUT + BPC), BF16)
        wb[:, :BPC * OUT] = (
            weights[banks].transpose(1, 0, 2).reshape(IN, BPC * OUT))
        wb[:, BPC * OUT:] = bias[banks].T                # [128, BPC]
        in_maps.append({"xs": xsT, "wb": wb})
    return in_maps


def kernel(tensor, bank_weights, bank_selections, weights, bias):
    tensor = np.asarray(tensor)
    bank_weights = np.asarray(bank_weights, dtype=np.float32)
    bank_selections = np.asarray(bank_selections)
    weights = np.asarray(weights)
    bias = np.asarray(bias)

    (group, caps, offs, CT, chunks,
     pair_core, pair_slot, xs_idx) = _routing_plan(bank_selections)
    nc = _build_program(caps, offs, CT, chunks)
    in_maps = _make_in_maps(tensor, bank_weights, bank_selections,
                            weights, bias, group, caps, offs, CT, xs_idx)

    nc.finalize()
    from concourse.bass_utils import run_bass_kernel_spmd
    try:
        res = run_bass_kernel_spmd(nc, in_maps, list(range(NCORES)))
    except Exception:
        # one retry: a previous crashed session can leave the accelerator in
        # a transient bad state that clears on the next dispatch
        import time
        time.sleep(2.0)
        res = run_bass_kernel_spmd(nc, in_maps, list(range(NCORES)))

    # reassemble per-core y panels -> Y [NCORES, CT, OUT] f32
    Y = np.empty((NCORES, CT, OUT), np.float32)
    for c in range(NCORES):
        for i, (c0, w, nb) in enumerate(chunks):
            Y[c, c0:c0 + w] = res.results[c][f"y{i}"].T.astype(np.float32)

    out = (Y[pair_core[:, 0], pair_slot[:, 0]] * bank_weights[:, 0:1] +
           Y[pair_core[:, 1], pair_slot[:, 1]] * bank_weights[:, 1:2])
    return out.astype(np.float32)
